# revision 6
# baseline (speedup 1.0000x reference)
"""Trainium2 Bass kernel for nn_BitsPredictor: teacher-forced LSTM bits predictor.

Strategy (data-parallel over batch, 8 NeuronCores, batch 256 -> 32/core):
  - All big matmuls in bf16 (PE streams 1 cycle/row bf16 vs 4 for fp32),
    fp32 PSUM accumulate, fp32 cell state.
  - Per-core state layout: partition = (S-quarter q, batch b) [4*32=128],
    free = (gate, s' in quarter) so LSTM cell math runs on full 128 lanes.
  - Recurrence step t computes gates = [h_{t-1}; inp_t] @ [Whh; Wih] + b as
    16 k-tiles x 4 column-groups of concurrent M=32 matmuls (tile_position
    col tiling); bias enters PSUM via a K=4 indicator matmul (start=True).
  - inp_t (teacher inputs: first = x@W1+b1 at t=0, then W4[target_ints]+b4)
    is precomputed transposed in SBUF; the one-hot embedding matmul runs on
    device from a host-built one-hot.
  - h_t is transposed back to stationary layout with 2 PE transposes/step,
    kept in a 3-deep SBUF ring and stashed to HBM for the final head matmul
    (pred = outputs @ W5 + b5, col-tiled over 4 steps/group).
  - log-softmax NLL loss is a scalar reduction of pred, done on host.
"""

import numpy as np
import ml_dtypes

import concourse.bass as bass
import concourse.mybir as mybir
import concourse.tile as tile
from concourse.bass_utils import run_bass_kernel_spmd
from concourse.masks import make_identity
from concourse.vector_clock import ScopedClock

# ---------------------------------------------------------------- shapes
B, IN, S, NBITS, K = 256, 2048, 1024, 512, 8
V = 2 ** K            # 256
T = NBITS // K        # 64
NC = 8                # cores
BL = B // NC          # 32 batch per core
P = 128
SQ = S // 4           # 256 per quarter
KT_IN = IN // P       # 16 k-tiles of x
KT_S = S // P         # 8 k-tiles of S
KT_BIG = 2 * KT_S     # 16 k-tiles of [h; inp]
AF = mybir.ActivationFunctionType
OP = mybir.AluOpType
BF16 = mybir.dt.bfloat16
F32 = mybir.dt.float32

_WAIT_CAP = 1


# ------------------------------------------------------- walrus workarounds
def _patched_drain_and_barrier(self, tick_clock, wait_clock):
    """Replacement for TileContext._drain_and_barrier: the pinned walrus build
    rejects instructions with more than one sync wait, and decorates InstDrain
    with its own per-queue waits.  Move the tail waits onto single-wait sync
    nops and use sem-only barriers."""
    nc = self.nc
    lead = nc.sync.nop(nofuse=True)
    wait_clock.add_sem_waits(lead.ins, ScopedClock({None: tick_clock.global_clock}))
    si = lead.ins.sync_info
    waits = list(si.on_wait) if (si and si.on_wait) else []
    if len(waits) > _WAIT_CAP:
        si.on_wait = waits[:_WAIT_CAP]
        for w in waits[_WAIT_CAP:]:
            nop = nc.sync.nop(nofuse=True)
            nsi = nop.ins.sync_info
            if nsi is None:
                nop.ins.sync_info = mybir.SyncInfo(on_wait=[w], on_update=[])
            else:
                nsi.on_wait = [w]
    nc.all_engine_barrier(sem_only=True)
    assert self.sems is not None
    popped = nc._tile_sem_poison_stack.pop()
    assert popped is self._sem_poison
    nc.clear_and_free_semaphores(list(self.sems.allocated().values()))
    nc.all_engine_barrier(sem_only=True)


tile.TileContext._drain_and_barrier = _patched_drain_and_barrier


def _split_waits(nc):
    """Hoist sem waits beyond _WAIT_CAP per instruction onto same-engine
    NoOps inserted immediately before the instruction."""
    n_added = 0
    for fn in nc.m.functions:
        for bb in fn.blocks:
            il = bb.instructions
            i = 0
            while i < len(il):
                inst = il[i]
                si = inst.sync_info
                waits = list(si.on_wait) if (si and si.on_wait) else []
                if len(waits) > _WAIT_CAP:
                    si.on_wait = waits[-_WAIT_CAP:]
                    pre = waits[:-_WAIT_CAP]
                    for j, w in enumerate(pre):
                        nop = mybir.InstNoOp(
                            name=nc.get_next_instruction_name(),
                            ins=[], outs=[], bass_nofuse=True,
                        )
                        nop.engine = inst.engine
                        nop.sync_info = mybir.SyncInfo(on_wait=[w], on_update=[])
                        il.insert(i + j, nop)
                        n_added += 1
                    i += len(pre)
                i += 1
    return n_added


# ------------------------------------------------------------ device program
def build_nc():
    nc = bass.Bass()

    # per-core external inputs (bf16 prepped on host)
    d_xT = nc.declare_dram_parameter("xT", [IN, BL], BF16, isOutput=False)
    d_oh = nc.declare_dram_parameter("oh", [V, (T - 1) * BL], BF16, isOutput=False)
    # replicated weights
    d_w = nc.declare_dram_parameter("wbig", [2 * S, 4, 4 * SQ], BF16, isOutput=False)
    d_w123 = nc.declare_dram_parameter("w123", [IN, 4, 3 * SQ], BF16, isOutput=False)
    d_w4 = nc.declare_dram_parameter("w4", [V, S], BF16, isOutput=False)
    d_w5 = nc.declare_dram_parameter("w5", [S, V], BF16, isOutput=False)
    d_bg = nc.declare_dram_parameter("biasG", [4, 4 * SQ], BF16, isOutput=False)
    d_bp = nc.declare_dram_parameter("biasP", [4, 3 * SQ], BF16, isOutput=False)
    d_b4c = nc.declare_dram_parameter("b4col", [P, KT_S], F32, isOutput=False)
    d_b5 = nc.declare_dram_parameter("b5r", [1, V], BF16, isOutput=False)
    d_ind = nc.declare_dram_parameter("ind", [4, P], BF16, isOutput=False)
    d_ones = nc.declare_dram_parameter("ones", [1, P], BF16, isOutput=False)
    # output
    d_pred = nc.declare_dram_parameter("pred", [T, BL, V], F32, isOutput=True)
    # internal stash of h_T tiles for the head phase
    d_hT = nc.dram_tensor("hT_stash", [T, P, KT_S, BL], BF16)

    with tile.TileContext(nc) as tc:
        with (
            tc.tile_pool(name="wpool", bufs=1) as wpool,
            tc.tile_pool(name="work", bufs=2) as work,
            tc.tile_pool(name="ring", bufs=3) as ring_pool,
        ):
            # ---- constants packed into arena tiles (saves per-tile padding)
            # bf16 arena columns: ident 128 | ind 128 | ones 128 | bg 1024 |
            #                     bp 768 | b5 256  (total 2432)
            arena = wpool.tile([P, 2432], BF16, name="arena")
            ident = arena[:, 0:P]
            make_identity(nc, ident)
            indt = arena[0:4, P:2 * P]
            nc.sync.dma_start(indt, d_ind[:, :])
            onest = arena[0:1, 2 * P:3 * P]
            nc.sync.dma_start(onest, d_ones[:, :])
            bg = arena[0:4, 384:384 + 1024]
            nc.sync.dma_start(bg, d_bg[:, :])
            bp = arena[0:4, 1408:1408 + 768]
            nc.sync.dma_start(bp, d_bp[:, :])
            b5r = arena[0:1, 2176:2176 + 256]
            nc.sync.dma_start(b5r, d_b5[:, :])
            # f32 arena: b4col 8 | c_t 256
            f32arena = wpool.tile([P, KT_S + SQ], F32, name="f32arena")
            b4c = f32arena[:, 0:KT_S]
            nc.sync.dma_start(b4c, d_b4c[:, :])
            xT = wpool.tile([P, KT_IN, BL], BF16, name="xT")
            nc.sync.dma_start(xT[:], d_xT.rearrange("(kt p) b -> p kt b", p=P))

            # teacher inputs, transposed: [128, kt, slot*32+b]; slot t+1 = inp_t
            teacher = wpool.tile([P, KT_S, (T + 1) * BL], BF16, name="teacher")

            # big recurrent weight [Whh; Wih], one tile per k-tile for precise deps
            wt = [
                wpool.tile([P, 4, 4 * SQ], BF16, name=f"wt{k}", tag=f"wt{k}")
                for k in range(KT_BIG)
            ]
            w_re = d_w.rearrange("(kt p) q n -> kt p q n", p=P)
            for k in list(range(KT_S, KT_BIG)) + list(range(KT_S)):
                nc.sync.dma_start(wt[k][:], w_re[k])

            # cell state
            c_t = f32arena[:, KT_S:]

            # ---------------- phase A: embedding + projections ----------------
            with (
                tc.tile_pool(name="phA", bufs=2) as phA,
                tc.tile_pool(name="psA", bufs=1, space="PSUM") as psA,
                tc.tile_pool(name="tpA", bufs=2, space="PSUM") as tpA,
            ):
                w4sb = phA.tile([P, 2, S], BF16, name="w4sb", bufs=1)
                nc.sync.dma_start(w4sb[:], d_w4.rearrange("(k2 p) n -> p k2 n", p=P))
                ohsb = phA.tile([P, 2, (T - 1) * BL], BF16, name="ohsb", bufs=1)
                nc.sync.dma_start(ohsb[:], d_oh.rearrange("(k2 p) n -> p k2 n", p=P))
                w123_re = d_w123.rearrange("(kt p) q n -> kt p q n", p=P)

                # embeddings: emb_T[ktile j] = (one_hot @ W4).T + b4, into teacher
                NE = (T - 1) * BL  # 2016
                for j in range(KT_S):
                    pe = psA.tile([P, 4, 512], F32, name="pe", tag="pe")
                    for k2 in range(2):
                        for c in range(4):
                            nc.tensor.matmul(
                                pe[:, c, 0:504],
                                lhsT=w4sb[:, k2, j * P:(j + 1) * P],
                                rhs=ohsb[:, k2, c * 504:(c + 1) * 504],
                                start=(k2 == 0),
                                stop=(k2 == 1),
                            )
                    nc.scalar.activation(
                        teacher[:, j, 2 * BL:], pe[:, :, 0:504], AF.Identity,
                        bias=b4c[:, j:j + 1],
                    )

                # projections first/h0/c0 = x @ [W1|W2|W3] + b, quarter layout
                pp = psA.tile([P, 3 * SQ], F32, name="pp", tag="pp")
                nc.tensor.matmul(pp[:, 0:512], lhsT=indt, rhs=bp[:, 0:512],
                                 start=True, stop=False)
                nc.tensor.matmul(pp[:, 512:768], lhsT=indt, rhs=bp[:, 512:768],
                                 start=True, stop=False)
                for kt in range(KT_IN):
                    w123t = phA.tile([P, 4, 3 * SQ], BF16, name="w123t", tag="w123t", bufs=2)
                    nc.sync.dma_start(w123t[:], w123_re[kt])
                    for q in range(4):
                        for lo, hi in ((0, 512), (512, 768)):
                            nc.tensor.matmul(
                                pp[q * BL:(q + 1) * BL, lo:hi],
                                lhsT=xT[:, kt, :],
                                rhs=w123t[:, q, lo:hi],
                                start=False,
                                stop=(kt == KT_IN - 1 and lo == 512),
                                tile_position=(0, q * BL),
                            )
                first_bf = work.tile([P, SQ], BF16, name="first_bf", tag="fb")
                nc.scalar.activation(first_bf[:], pp[:, 0:SQ], AF.Copy)
                h0_bf = work.tile([P, SQ], BF16, name="h0_bf", tag="hb")
                nc.scalar.activation(h0_bf[:], pp[:, SQ:2 * SQ], AF.Copy)
                nc.vector.tensor_copy(c_t[:, :], pp[:, 2 * SQ:3 * SQ])

                # transpose h0 -> ring slot, first -> teacher slot 1
                ring0 = ring_pool.tile([P, KT_S, BL], BF16, name="ring0", tag="hT")
                for j in range(2):
                    tp = tpA.tile([P, P], BF16, name="tp", tag="tpA")
                    nc.tensor.transpose(tp[:], h0_bf[:, j * P:(j + 1) * P], ident)
                    nc.vector.tensor_copy(
                        ring0[:, j:KT_S:2, :],
                        tp[:].rearrange("p (q b) -> p q b", b=BL),
                    )
                for j in range(2):
                    tp = tpA.tile([P, P], BF16, name="tp2", tag="tpA")
                    nc.tensor.transpose(tp[:], first_bf[:, j * P:(j + 1) * P], ident)
                    nc.vector.tensor_copy(
                        teacher[:, j:KT_S:2, BL:2 * BL],
                        tp[:].rearrange("p (q b) -> p q b", b=BL),
                    )

            # ---------------- phase B: recurrence ----------------
            with (
                tc.tile_pool(name="psB", bufs=2, space="PSUM") as psB,
                tc.tile_pool(name="tpB", bufs=2, space="PSUM") as tpB,
            ):
                ring_prev = ring0
                h_prev = None  # h tile of previous step (sbuf bf16)
                for t in range(T):
                    ps = psB.tile([P, 4 * SQ], F32, name="gates", tag="gates")
                    # bias first (start=True over all 128 partitions per bank)
                    for c in range(2):
                        nc.tensor.matmul(
                            ps[:, c * 512:(c + 1) * 512], lhsT=indt,
                            rhs=bg[:, c * 512:(c + 1) * 512],
                            start=True, stop=False,
                        )
                    # inp part: k-tiles 8..15, stationary from teacher slot t+1
                    for kt in range(KT_S):
                        st = teacher[:, kt, (t + 1) * BL:(t + 2) * BL]
                        for c in range(2):
                            for g in range(4):
                                nc.tensor.matmul(
                                    ps[g * BL:(g + 1) * BL, c * 512:(c + 1) * 512],
                                    lhsT=st,
                                    rhs=wt[KT_S + kt][:, g, c * 512:(c + 1) * 512],
                                    start=False, stop=False,
                                    tile_position=(0, g * BL),
                                )
                    # transpose h_{t-1} (overlaps with the inp MMs above)
                    if t > 0:
                        ring_prev = ring_pool.tile(
                            [P, KT_S, BL], BF16, name="ringt", tag="hT"
                        )
                        for j in range(2):
                            tp = tpB.tile([P, P], BF16, name="tpB", tag="tpB")
                            nc.tensor.transpose(
                                tp[:], h_prev[:, j * P:(j + 1) * P], ident
                            )
                            nc.vector.tensor_copy(
                                ring_prev[:, j:KT_S:2, :],
                                tp[:].rearrange("p (q b) -> p q b", b=BL),
                            )
                        nc.sync.dma_start(d_hT[t - 1], ring_prev[:])
                    # h part: k-tiles 0..7, stationary = h_T(t-1)
                    for kt in range(KT_S):
                        for c in range(2):
                            for g in range(4):
                                nc.tensor.matmul(
                                    ps[g * BL:(g + 1) * BL, c * 512:(c + 1) * 512],
                                    lhsT=ring_prev[:, kt, :],
                                    rhs=wt[kt][:, g, c * 512:(c + 1) * 512],
                                    start=False,
                                    stop=(kt == KT_S - 1 and c == 1),
                                    tile_position=(0, g * BL),
                                )
                    # cell math; gate order (i, f, o, g~) along free dim
                    acts = work.tile([P, 4 * SQ], F32, name="acts", tag="acts")
                    nc.scalar.activation(acts[:, 0:3 * SQ], ps[:, 0:3 * SQ], AF.Sigmoid)
                    nc.scalar.activation(acts[:, 3 * SQ:], ps[:, 3 * SQ:], AF.Tanh)
                    t1 = work.tile([P, SQ], F32, name="t1", tag="t1")
                    nc.vector.tensor_tensor(t1[:], acts[:, SQ:2 * SQ], c_t[:, :], OP.mult)
                    nc.vector.tensor_tensor(
                        c_t[:, :], acts[:, 0:SQ], acts[:, 3 * SQ:], OP.mult
                    )
                    nc.vector.tensor_tensor(c_t[:, :], c_t[:, :], t1[:], OP.add)
                    tch = work.tile([P, SQ], F32, name="tch", tag="tch")
                    nc.scalar.activation(tch[:], c_t[:, :], AF.Tanh)
                    h_prev = work.tile([P, SQ], BF16, name="h", tag="h")
                    nc.vector.tensor_tensor(
                        h_prev[:], acts[:, 2 * SQ:3 * SQ], tch[:], OP.mult
                    )
                # final h transpose + stash
                ring_last = ring_pool.tile([P, KT_S, BL], BF16, name="ringl", tag="hT")
                for j in range(2):
                    tp = tpB.tile([P, P], BF16, name="tpBl", tag="tpB")
                    nc.tensor.transpose(tp[:], h_prev[:, j * P:(j + 1) * P], ident)
                    nc.vector.tensor_copy(
                        ring_last[:, j:KT_S:2, :],
                        tp[:].rearrange("p (q b) -> p q b", b=BL),
                    )
                nc.sync.dma_start(d_hT[T - 1], ring_last[:])

            # ---------------- phase C: head pred = outputs @ W5 + b5 ----------
            with (
                tc.tile_pool(name="phC", bufs=3) as phC,
                tc.tile_pool(name="psC", bufs=4, space="PSUM") as psC,
            ):
                w5sb = phC.tile([P, KT_S, V], BF16, name="w5sb", bufs=1)
                nc.sync.dma_start(w5sb[:], d_w5.rearrange("(k p) n -> p k n", p=P))
                for J in range(T // 4):
                    hj = phC.tile([P, 4, KT_S, BL], BF16, name="hj", tag="hj")
                    nc.sync.dma_start(
                        hj[:], d_hT[4 * J:4 * (J + 1)].rearrange("g p k b -> p g k b")
                    )
                    ph = psC.tile([P, V], F32, name="ph", tag="ph")
                    nc.tensor.matmul(ph[:], lhsT=onest, rhs=b5r,
                                     start=True, stop=False)
                    for g in range(4):
                        for k in range(KT_S):
                            nc.tensor.matmul(
                                ph[g * BL:(g + 1) * BL, :],
                                lhsT=hj[:, g, k, :],
                                rhs=w5sb[:, k, :],
                                start=False, stop=(k == KT_S - 1),
                                tile_position=(0, g * BL),
                            )
                    po = phC.tile([P, V], F32, name="po", tag="po")
                    nc.vector.tensor_copy(po[:], ph[:])
                    nc.sync.dma_start(
                        d_pred[4 * J:4 * (J + 1), :, :].rearrange("g b n -> (g b) n"),
                        po[:],
                    )
    _split_waits(nc)
    return nc


# ------------------------------------------------------------- host wrapper
_cached = {}


def _get_nc():
    if "nc" not in _cached:
        _cached["nc"] = build_nc()
    return _cached["nc"]


def _prep_inputs(x, target_bits, W1, b1, W2, b2, W3, b3, W4, b4, W5, b5,
                 Wih, Whh, bih, bhh):
    """Host-side input preparation -> per-core input maps (bf16 layouts)."""
    bf = ml_dtypes.bfloat16
    f32 = np.float32
    x = np.asarray(x, f32)
    tb = np.asarray(target_bits)

    # bit-pack LSB-first -> ints [B, T]
    powers = (1 << np.arange(K)).astype(np.int64)
    ints = (np.maximum(tb, 0).reshape(B, T, K).astype(np.int64) * powers).sum(-1)
    ints = ints.astype(np.int32)

    perm = [0, 1, 3, 2]  # device gate order (i, f, o, g) from pytorch (i, f, g, o)

    def gate_quarter(w):  # [rows, 4S] -> [rows, 4 quarters, 4 gates * SQ]
        w = np.asarray(w, f32).reshape(-1, 4, 4, SQ)       # [r, gate, q, s']
        w = w[:, perm][:, :, :, :]                          # reorder gates
        return np.ascontiguousarray(w.transpose(0, 2, 1, 3)).reshape(-1, 4, 4 * SQ)

    wbig = gate_quarter(np.concatenate([np.asarray(Whh, f32),
                                        np.asarray(Wih, f32)], 0)).astype(bf)
    biasG = gate_quarter((np.asarray(bih, f32) + np.asarray(bhh, f32))[None, :])[0]
    biasG = biasG.astype(bf)                                # [4, 4*SQ]

    w123 = np.stack([np.asarray(W1, f32), np.asarray(W2, f32),
                     np.asarray(W3, f32)], 1)               # [IN, 3, S]
    w123 = w123.reshape(IN, 3, 4, SQ).transpose(0, 2, 1, 3) # [IN, q, m, s']
    w123 = np.ascontiguousarray(w123).reshape(IN, 4, 3 * SQ).astype(bf)
    biasP = np.stack([np.asarray(b1, f32), np.asarray(b2, f32),
                      np.asarray(b3, f32)], 0)              # [3, S]
    biasP = biasP.reshape(3, 4, SQ).transpose(1, 0, 2)      # [q, m, s']
    biasP = np.ascontiguousarray(biasP).reshape(4, 3 * SQ).astype(bf)

    w4 = np.asarray(W4, f32).astype(bf)                     # [V, S]
    b4col = np.asarray(b4, f32).reshape(KT_S, P).T.copy()   # [P, KT_S]
    w5 = np.asarray(W5, f32).astype(bf)                     # [S, V]
    b5r = np.asarray(b5, f32).reshape(1, V).astype(bf)

    ind = np.zeros((4, P), f32)
    for q in range(4):
        ind[q, q * BL:(q + 1) * BL] = 1.0
    ind = ind.astype(bf)
    ones = np.ones((1, P), f32).astype(bf)

    in_maps = []
    for c in range(NC):
        sl = slice(c * BL, (c + 1) * BL)
        xT = np.ascontiguousarray(x[sl].T).astype(bf)       # [IN, BL]
        # one-hot^T [V, (T-1)*BL], col index = t*BL + b, for inp_{t+1}=emb[t]
        ii = ints[sl, :T - 1]                               # [BL, T-1]
        oh = np.zeros((V, (T - 1) * BL), f32)
        tt, bb2 = np.meshgrid(np.arange(T - 1), np.arange(BL), indexing="ij")
        oh[ii.T.ravel(), (tt * BL + bb2).ravel()] = 1.0
        oh = oh.astype(bf)
        in_maps.append({
            "xT": xT, "oh": oh, "wbig": wbig, "w123": w123, "w4": w4,
            "w5": w5, "biasG": biasG, "biasP": biasP, "b4col": b4col,
            "b5r": b5r, "ind": ind, "ones": ones,
        })
    return in_maps, ints


def _loss_from_pred(pred, ints):
    """loss = mean(-log_softmax(pred)[target]) in fp32, matching jax."""
    p = pred.astype(np.float32)
    m = p.max(axis=-1, keepdims=True)
    lse = m + np.log(np.exp(p - m).sum(axis=-1, keepdims=True))
    tgt = np.take_along_axis(p, ints[..., None].astype(np.int64), axis=-1)
    nll = (lse - tgt)[..., 0]
    return np.float32(nll.mean(dtype=np.float32))


def kernel(x, target_bits, temperature, W1, b1, W2, b2, W3, b3, W4, b4,
           W5, b5, Wih, Whh, bih, bhh):
    del temperature  # unused on the teacher-forcing path
    in_maps, ints = _prep_inputs(x, target_bits, W1, b1, W2, b2, W3, b3,
                                 W4, b4, W5, b5, Wih, Whh, bih, bhh)
    nc = _get_nc()
    res = run_bass_kernel_spmd(nc, in_maps, list(range(NC)))
    pred = np.concatenate(
        [res.results[c]["pred"].transpose(1, 0, 2) for c in range(NC)], axis=0)
    loss = _loss_from_pred(pred, ints)
    return pred, loss
